# revision 3
# baseline (speedup 1.0000x reference)
"""GCN (3-layer, PyG-style) forward on 8 Trainium2 NeuronCores.

Math restructuring
------------------
reference:
  h1 = relu(Anorm @ x @ W1 + b1)          (Anorm includes self loops + sym norm)
  h2 = relu(Anorm @ h1 @ W2 + b2)
  h3 = Anorm @ h2 @ W3 + b3
  out = segment_mean(h3, batch) @ Wlin + blin

Because GCNConv aggregation and the weight matmul commute, and pooling is
linear, this is equivalent to:
  agg1 = Anorm @ x                        # [N,2]  (tiny -> host)
  h1   = relu(agg1 @ W1 + b1)             # rank-3 structure: per-EDGE on device
  g2   = Anorm @ h1                       # one-hot scatter matmul on device
  h2   = relu(g2 @ W2 + b2)               # dense matmul on device
  pg3[g] = sum_e norm_e * h2[row_e] * [batch[col_e]==g]   # dense T.T@h2 on device
  out  = ((pg3 @ W3 + cnt*b3)/max(cnt,1)) @ Wlin + blin   # [128,1024] -> host

Sharding: nodes (and L2 edges by dst / L3 edges by src) are partitioned into
8 contiguous blocks of 12500.  Every core runs the same program (SPMD) on its
own edge arrays, padded to identical tile counts.  Per-core output is a
partial pg3 [128,1024]; the host sums them (the "all-reduce").
"""

import numpy as np

LAST_RESULTS = None  # set by kernel() for test harness introspection

N_NODES = 100000
N_EDGES = 400000
G = 128
FIN = 2
H = 1024
N_CORES = 8
NPC = N_NODES // N_CORES          # 12500 nodes per core
P = 128
NW = (NPC + P - 1) // P           # 98 windows (last has 84 dsts)


def _host_prep(x, edge_index, batch):
    """All O(E) index work in numpy; returns per-core device arrays."""
    x = np.asarray(x, dtype=np.float32)
    ei = np.asarray(edge_index).astype(np.int64)
    batch = np.asarray(batch).astype(np.int64)
    n = N_NODES

    loops = np.arange(n, dtype=np.int64)
    row = np.concatenate([ei[0], loops])
    col = np.concatenate([ei[1], loops])

    deg = np.bincount(col, minlength=n).astype(np.float64)
    dis = np.where(deg > 0, 1.0 / np.sqrt(np.maximum(deg, 1.0)), 0.0)
    norm = (dis[row] * dis[col]).astype(np.float64)

    # layer-1 aggregation (FIN=2) on host
    agg1 = np.empty((n, FIN), dtype=np.float32)
    for f in range(FIN):
        agg1[:, f] = np.bincount(
            col, weights=norm * x[row, f].astype(np.float64), minlength=n
        ).astype(np.float32)

    norm = norm.astype(np.float32)

    # ---- L2 edge partition by destination core, sorted by col ----
    core_of = col // NPC
    order = np.argsort(col, kind="stable")
    row_s, col_s, norm_s = row[order], col[order], norm[order]
    core_s = core_of[order]

    # window of each edge (within its core), and counts per (core, window)
    col_local = col_s - core_s * NPC
    win = col_local // P                       # 0..NW-1
    cw = core_s * NW + win
    cw_counts = np.bincount(cw, minlength=N_CORES * NW).reshape(N_CORES, NW)
    tiles_per_cw = (cw_counts + P - 1) // P
    T_w = tiles_per_cw.max(axis=0)             # shared tile count per window
    base_tile = np.concatenate([[0], np.cumsum(T_w)])
    TT = int(base_tile[-1])                    # total edge tiles per core

    # position of each edge within its (core, window) run
    cw_starts = np.concatenate([[0], np.cumsum(cw_counts.reshape(-1))])
    idx_in_cw = np.arange(len(col_s)) - cw_starts[cw]
    tile_in_w = idx_in_cw // P
    pos = idx_in_cw % P
    tile_global = base_tile[win] + tile_in_w   # 0..TT-1 within the core

    # per-core device arrays
    aT = np.zeros((N_CORES, 4, TT * P), dtype=np.float32)
    S = np.zeros((N_CORES, TT * P, P), dtype=np.float32)
    slot = tile_global * P + pos
    dst_in_win = col_local - win * P
    c = core_s
    aT[c, 0, slot] = agg1[row_s, 0]
    aT[c, 1, slot] = agg1[row_s, 1]
    aT[c, 2, slot] = 1.0
    S[c, slot, dst_in_win] = norm_s

    # ---- L3: T matrix, edges partitioned by source ----
    gcol = batch[col]                          # graph of each edge's dst
    Tmat = np.bincount(
        row * G + gcol, weights=norm.astype(np.float64), minlength=n * G
    ).astype(np.float32).reshape(n, G)
    # reshape to per-core [NW*P, G] with zero padding rows
    Tpad = np.zeros((N_CORES, NW * P, G), dtype=np.float32)
    Tpad[:, :NPC, :] = Tmat.reshape(N_CORES, NPC, G)

    cnt = np.bincount(batch, minlength=G).astype(np.float32)
    return agg1, aT, S, Tpad, cnt, T_w, TT


def _build_device_program(TT, T_w, nw=NW):
    import concourse.mybir as mybir
    import concourse.tile as tile
    from concourse import bacc
    from concourse.masks import make_identity

    f32 = mybir.dt.float32
    nc = bacc.Bacc(None, target_bir_lowering=False, debug=False)

    aT_d = nc.dram_tensor("aT", [4, TT * P], f32, kind="ExternalInput")
    S_d = nc.dram_tensor("S", [TT, P, P], f32, kind="ExternalInput")
    T_d = nc.dram_tensor("T", [NW, P, G], f32, kind="ExternalInput")
    W1b_d = nc.dram_tensor("W1b", [4, H], f32, kind="ExternalInput")
    W2_d = nc.dram_tensor("W2", [8, P, H], f32, kind="ExternalInput")
    b2_d = nc.dram_tensor("b2", [1, H], f32, kind="ExternalInput")
    out_d = nc.dram_tensor("pg3", [G, H], f32, kind="ExternalOutput")

    CH = 32  # aT tiles per staged chunk

    with tile.TileContext(nc) as tc:
        with (
            tc.tile_pool(name="const", bufs=1) as cst,
            tc.tile_pool(name="sa", bufs=2) as sa,
            tc.tile_pool(name="sS", bufs=4) as sS,
            tc.tile_pool(name="smsg", bufs=3) as smsg,
            tc.tile_pool(name="sg2s", bufs=2) as sg2s,
            tc.tile_pool(name="sg2T", bufs=2) as sg2T,
            tc.tile_pool(name="sh2", bufs=2) as sh2,
            tc.tile_pool(name="sT", bufs=2) as sT,
            tc.tile_pool(name="zp", bufs=3, space="PSUM") as zp,
            tc.tile_pool(name="g2p", bufs=1, space="PSUM") as g2p,
            tc.tile_pool(name="hp", bufs=3, space="PSUM") as hp,
        ):
            Relu = mybir.ActivationFunctionType.Relu
            Copy = mybir.ActivationFunctionType.Copy

            W1b = cst.tile([4, H], f32, tag="W1b")
            nc.sync.dma_start(W1b[:], W1b_d[:])
            W2s = cst.tile([P, 8, H], f32, tag="W2s")
            nc.sync.dma_start(W2s[:], W2_d[:].rearrange("c p f -> p c f"))
            b2s = cst.tile([1, H], f32, tag="b2s")
            nc.sync.dma_start(b2s[:], b2_d[:])
            ones1 = cst.tile([1, P], f32, tag="ones1")
            nc.vector.memset(ones1[:], 1.0)
            ident = cst.tile([P, P], f32, tag="ident")
            make_identity(nc, ident[:])
            pg3s = cst.tile([G, H], f32, tag="pg3s")
            nc.vector.memset(pg3s[:], 0.0)

            aTc = None
            for w in range(nw):
                Tt = sT.tile([P, G], f32, tag="Tt")
                nc.sync.dma_start(Tt[:], T_d[w])

                g2 = g2p.tile([P, H], f32, tag="g2")
                nt = int(T_w[w])
                for t in range(nt):
                    tg = int(np.sum(T_w[:w])) + t
                    if tg % CH == 0:
                        aTc = sa.tile([4, CH * P], f32, tag="aTc")
                        hi = min((tg + CH) * P, TT * P)
                        nc.sync.dma_start(
                            aTc[:, : hi - tg * P], aT_d[:, tg * P : hi]
                        )
                    off = (tg % CH) * P
                    Ss = sS.tile([P, P], f32, tag="Ss")
                    nc.sync.dma_start(Ss[:], S_d[tg])

                    zA = zp.tile([P, 512], f32, tag="z")
                    zB = zp.tile([P, 512], f32, tag="z")
                    lhs_a = aTc[:, off : off + P]
                    nc.tensor.matmul(zA[:], lhs_a, W1b[:, :512], start=True, stop=True)
                    nc.tensor.matmul(zB[:], lhs_a, W1b[:, 512:], start=True, stop=True)
                    msg = smsg.tile([P, H], f32, tag="msg")
                    nc.scalar.activation(msg[:, :512], zA[:], Relu)
                    nc.vector.tensor_scalar_max(msg[:, 512:], zB[:], 0.0)

                    nc.tensor.matmul(
                        g2[:, :512], Ss[:], msg[:, :512],
                        start=(t == 0), stop=(t == nt - 1),
                    )
                    nc.tensor.matmul(
                        g2[:, 512:], Ss[:], msg[:, 512:],
                        start=(t == 0), stop=(t == nt - 1),
                    )

                # g2 [128 dst, 1024] -> transpose to g2T [1024 fin, 128 dst]
                g2s = sg2s.tile([P, H], f32, tag="g2s")
                nc.vector.tensor_copy(g2s[:, :512], g2[:, :512])
                nc.scalar.activation(g2s[:, 512:], g2[:, 512:], Copy)
                g2T = sg2T.tile([P, 8, P], f32, tag="g2T")
                for j in range(8):
                    tp = zp.tile([P, P], f32, tag="z")
                    nc.tensor.transpose(tp[:], g2s[:, j * P : (j + 1) * P], ident[:])
                    if j % 2 == 0:
                        nc.vector.tensor_copy(g2T[:, j], tp[:])
                    else:
                        nc.scalar.activation(g2T[:, j], tp[:], Copy)

                # h2 = relu(g2 @ W2 + b2), then pg3 += T.T @ h2
                h2s = sh2.tile([P, H], f32, tag="h2s")
                for half in range(2):
                    lo = half * 512
                    h2p = hp.tile([P, 512], f32, tag="h")
                    for j in range(8):
                        nc.tensor.matmul(
                            h2p[:], g2T[:, j], W2s[:, j, lo : lo + 512],
                            start=(j == 0), stop=False,
                        )
                    nc.tensor.matmul(
                        h2p[:], ones1[:1, :], b2s[:1, lo : lo + 512],
                        start=False, stop=True,
                    )
                    if half == 0:
                        nc.scalar.activation(h2s[:, lo : lo + 512], h2p[:], Relu)
                    else:
                        nc.vector.tensor_scalar_max(h2s[:, lo : lo + 512], h2p[:], 0.0)

                    cp = hp.tile([P, 512], f32, tag="h")
                    nc.tensor.matmul(
                        cp[:], Tt[:], h2s[:, lo : lo + 512], start=True, stop=True
                    )
                    nc.vector.tensor_add(
                        pg3s[:, lo : lo + 512], pg3s[:, lo : lo + 512], cp[:]
                    )

            nc.sync.dma_start(out_d[:], pg3s[:])

    nc.finalize()
    return nc


def kernel(x, W1, b1, W2, b2, W3, b3, Wlin, blin, edge_index, batch, num_graphs):
    from concourse.bass_utils import run_bass_kernel_spmd

    x = np.asarray(x, dtype=np.float32)
    W1 = np.asarray(W1, dtype=np.float32)
    b1 = np.asarray(b1, dtype=np.float32)
    W2 = np.asarray(W2, dtype=np.float32)
    b2 = np.asarray(b2, dtype=np.float32)
    W3 = np.asarray(W3, dtype=np.float32)
    b3 = np.asarray(b3, dtype=np.float32)
    Wlin = np.asarray(Wlin, dtype=np.float32)
    blin = np.asarray(blin, dtype=np.float32)

    agg1, aT, S, Tpad, cnt, T_w, TT = _host_prep(x, edge_index, batch)

    nc = _build_device_program(TT, T_w)

    W1b = np.zeros((4, H), dtype=np.float32)
    W1b[:2] = W1
    W1b[2] = b1
    W2r = np.ascontiguousarray(W2.reshape(8, P, H))
    b2r = b2.reshape(1, H).astype(np.float32)

    in_maps = [
        {
            "aT": np.ascontiguousarray(aT[c]),
            "S": np.ascontiguousarray(S[c].reshape(TT, P, P)),
            "T": np.ascontiguousarray(Tpad[c].reshape(NW, P, G)),
            "W1b": W1b,
            "W2": W2r,
            "b2": b2r,
        }
        for c in range(N_CORES)
    ]
    res = run_bass_kernel_spmd(nc, in_maps, core_ids=list(range(N_CORES)))
    global LAST_RESULTS
    LAST_RESULTS = res
    pg3 = np.zeros((G, H), dtype=np.float64)
    for r in res.results:
        pg3 += r["pg3"].astype(np.float64)
    pg3 = pg3.astype(np.float32)

    pooled = (pg3 @ W3 + cnt[:, None] * b3[None, :]) / np.maximum(cnt, 1.0)[:, None]
    out = pooled @ Wlin + blin[None, :]
    return out.astype(np.float32)



# revision 8
# speedup vs baseline: 3.9691x; 3.9691x over previous
"""GCN (3-layer, PyG-style) forward on 8 Trainium2 NeuronCores.

Math restructuring
------------------
reference:
  h1 = relu(Anorm @ x @ W1 + b1)          (Anorm includes self loops + sym norm)
  h2 = relu(Anorm @ h1 @ W2 + b2)
  h3 = Anorm @ h2 @ W3 + b3
  out = segment_mean(h3, batch) @ Wlin + blin

Because GCNConv aggregation and the weight matmul commute, and pooling is
linear, this is equivalent to:
  agg1 = Anorm @ x                        # [N,2]  (tiny -> host)
  msg_e = relu(norm_e * (agg1[src_e] @ W1 + b1))    # per-edge (norm>0 commutes
                                                    #  through relu)
  g2   = scatter-sum msg to dst           # exact one-hot matmul on device
  h2   = relu(g2 @ W2 + b2)               # dense matmul on device
  pg3[g] = sum_n T[n,g] * h2[n]           # T[n,g] = sum of norm over n's
                                          #  out-edges into graph g
  out  = ((pg3 @ W3 + cnt*b3)/max(cnt,1)) @ Wlin + blin   # [128,1024] -> host

Sharding: nodes are LPT bin-packed into 8 cores x 98 windows of 128 slots so
that each (core, window) bin holds ~638 incident edges (load-balanced).  Every
core runs the same program (SPMD) on its own edge arrays, padded to identical
tile counts.  Per-core output is a partial pg3 [128,1024]; the host sums them
(the "all-reduce").

All matmul operands are bf16 (single-pass PE, FWL weight loads); PSUM
accumulation is fp32.  The aggregation runs in "dual form" (g2T = msg.T @ S
chunk-wise) so no PE transposes are needed before the W2 matmul.
"""

import numpy as np

LAST_RESULTS = None  # set by kernel() for test harness introspection

N_NODES = 100000
N_EDGES = 400000
G = 128
FIN = 2
H = 1024
N_CORES = 8
P = 128
NW = 98                      # windows per core (98*128 = 12544 >= 12500 slots)
NBINS = N_CORES * NW


def _lpt_pack(wgt):
    """Assign each node to one of 784 (core,window) bins, balancing total
    edge weight per bin with a <=128 nodes/bin cap.  Returns (bin_of, slot_of).
    """
    import heapq

    n = len(wgt)
    order = np.argsort(-wgt, kind="stable")
    heap = [(0, 0, b) for b in range(NBINS)]
    heapq.heapify(heap)
    bin_of = np.empty(n, dtype=np.int64)
    slot_of = np.empty(n, dtype=np.int64)
    w_arr = wgt.tolist()
    for idx in order.tolist():
        while True:
            load, count, b = heapq.heappop(heap)
            if count < P:
                break
        bin_of[idx] = b
        slot_of[idx] = count
        heapq.heappush(heap, (load + w_arr[idx], count + 1, b))
    return bin_of, slot_of


def _host_prep(x, edge_index, batch):
    """All O(E) index work in numpy; returns per-core device arrays."""
    import ml_dtypes

    bf16 = ml_dtypes.bfloat16
    x = np.asarray(x, dtype=np.float32)
    ei = np.asarray(edge_index).astype(np.int64)
    batch = np.asarray(batch).astype(np.int64)
    n = N_NODES

    loops = np.arange(n, dtype=np.int64)
    row = np.concatenate([ei[0], loops])
    col = np.concatenate([ei[1], loops])

    deg = np.bincount(col, minlength=n).astype(np.float64)
    dis = np.where(deg > 0, 1.0 / np.sqrt(np.maximum(deg, 1.0)), 0.0)
    norm = dis[row] * dis[col]                     # fp64

    # layer-1 aggregation (FIN=2) on host
    agg1 = np.empty((n, FIN), dtype=np.float64)
    for f in range(FIN):
        agg1[:, f] = np.bincount(
            col, weights=norm * x[row, f].astype(np.float64), minlength=n
        )

    # ---- node -> (core, window, slot) via LPT packing on indegree+1 ----
    wgt = np.bincount(col, minlength=n)            # includes the self loop
    bin_raw, slot_of = _lpt_pack(wgt)
    # deal bins to (core, window) so similar loads share a window
    loads = np.zeros(NBINS, dtype=np.int64)
    np.add.at(loads, bin_raw, wgt)
    deal = np.argsort(-loads, kind="stable")       # deal[k] = raw bin id
    bin_rank = np.empty(NBINS, dtype=np.int64)
    bin_rank[deal] = np.arange(NBINS)
    rank = bin_rank[bin_raw]                       # 0..783, sorted by load
    node_w = rank // N_CORES                       # window 0..97
    node_c = rank % N_CORES                        # core 0..7

    # ---- edges ordered by (dst core, dst window) ----
    e_rank = rank[col]
    order = np.argsort(e_rank, kind="stable")
    row_s, col_s = row[order], col[order]
    norm_s = norm[order]
    rank_s = e_rank[order]
    c_s = rank_s % N_CORES
    w_s = rank_s // N_CORES

    cnts = np.bincount(e_rank, minlength=NBINS)    # indexed by rank = w*8 + c
    cw_load = cnts.reshape(NW, N_CORES).T          # [core, window]
    T_w = ((cw_load.max(axis=0) + P - 1) // P).astype(np.int64)   # per window
    base_tile = np.concatenate([[0], np.cumsum(T_w)])
    TT = int(base_tile[-1])

    starts = np.concatenate([[0], np.cumsum(cnts)])
    idx_in_bin = np.arange(len(col_s)) - starts[rank_s]
    tile_g = base_tile[w_s] + idx_in_bin // P
    slot = tile_g * P + idx_in_bin % P

    # per-core device arrays (norm folded into aT; S is exact one-hot)
    aT = np.zeros((N_CORES, 4, TT * P), dtype=np.float32)
    S = np.zeros((N_CORES, TT * P, P), dtype=bf16)
    aT[c_s, 0, slot] = (agg1[row_s, 0] * norm_s).astype(np.float32)
    aT[c_s, 1, slot] = (agg1[row_s, 1] * norm_s).astype(np.float32)
    aT[c_s, 2, slot] = norm_s.astype(np.float32)
    S[c_s, slot, slot_of[col_s]] = bf16(1.0)

    # ---- L3: T matrix rows permuted to node home slots ----
    gcol = batch[col]                              # graph of each edge's dst
    Tmat = np.bincount(
        row * G + gcol, weights=norm, minlength=n * G
    ).astype(np.float32).reshape(n, G)
    Tpad = np.zeros((N_CORES, NW * P, G), dtype=bf16)
    Tpad[node_c, node_w * P + slot_of] = Tmat.astype(bf16)

    cnt = np.bincount(batch, minlength=G).astype(np.float32)
    return (agg1.astype(np.float32), aT.astype(bf16), S, Tpad, cnt, T_w, TT,
            base_tile)


def _build_device_program(TT, T_w, base_tile, nw=NW):
    import concourse.mybir as mybir
    import concourse.tile as tile
    from concourse import bacc

    f32 = mybir.dt.float32
    bf16 = mybir.dt.bfloat16
    nc = bacc.Bacc(None, target_bir_lowering=False, debug=False)

    aT_d = nc.dram_tensor("aT", [4, TT * P], bf16, kind="ExternalInput")
    S_d = nc.dram_tensor("S", [TT, P, P], bf16, kind="ExternalInput")
    T_d = nc.dram_tensor("T", [NW, P, G], bf16, kind="ExternalInput")
    W1b_d = nc.dram_tensor("W1b", [4, H], bf16, kind="ExternalInput")
    W2_d = nc.dram_tensor("W2", [8, P, H], bf16, kind="ExternalInput")
    b2_d = nc.dram_tensor("b2", [1, H], bf16, kind="ExternalInput")
    out_d = nc.dram_tensor("pg3", [G, H], f32, kind="ExternalOutput")

    CH = 32  # aT tiles per staged chunk

    with tile.TileContext(nc) as tc:
        with (
            tc.tile_pool(name="const", bufs=1) as cst,
            tc.tile_pool(name="sa", bufs=2) as sa,
            tc.tile_pool(name="sS", bufs=14) as sS,
            tc.tile_pool(name="smsg", bufs=14) as smsg,
            tc.tile_pool(name="sg2T", bufs=2) as sg2T,
            tc.tile_pool(name="sh2", bufs=2) as sh2,
            tc.tile_pool(name="sT", bufs=2) as sT,
            tc.tile_pool(name="zp", bufs=4, space="PSUM") as zp,
            tc.tile_pool(name="gp", bufs=4, space="PSUM") as gp,
        ):
            Relu = mybir.ActivationFunctionType.Relu
            Copy = mybir.ActivationFunctionType.Copy

            W1b = cst.tile([4, H], bf16, tag="W1b")
            nc.sync.dma_start(W1b[:], W1b_d[:])
            W2s = cst.tile([P, 8, H], bf16, tag="W2s")
            nc.sync.dma_start(W2s[:], W2_d[:].rearrange("c p f -> p c f"))
            b2s = cst.tile([1, H], bf16, tag="b2s")
            nc.sync.dma_start(b2s[:], b2_d[:])
            ones1 = cst.tile([1, P], bf16, tag="ones1")
            nc.vector.memset(ones1[:], 1.0)
            pg3s = cst.tile([G, H], f32, tag="pg3s")
            nc.vector.memset(pg3s[:], 0.0)

            aTc = None
            for w in range(nw):
                Tt = sT.tile([P, G], bf16, tag="Tt")
                nc.sync.dma_start(Tt[:], T_d[w])

                nt = int(T_w[w])
                msgs, Sss = [], []
                for t in range(nt):
                    tg = int(base_tile[w]) + t
                    if tg % CH == 0:
                        aTc = sa.tile([4, CH * P], bf16, tag="aTc")
                        hi = min((tg + CH) * P, TT * P)
                        nc.sync.dma_start(
                            aTc[:, : hi - tg * P], aT_d[:, tg * P : hi]
                        )
                    off = (tg % CH) * P
                    Ss = sS.tile([P, P], bf16, tag="Ss")
                    nc.sync.dma_start(Ss[:], S_d[tg])

                    zA = zp.tile([P, 512], f32, tag="z")
                    zB = zp.tile([P, 512], f32, tag="z")
                    lhs_a = aTc[:, off : off + P]
                    nc.tensor.matmul(zA[:], lhs_a, W1b[:, :512], start=True, stop=True)
                    nc.tensor.matmul(zB[:], lhs_a, W1b[:, 512:], start=True, stop=True)
                    msg = smsg.tile([P, H], bf16, tag="msg")
                    nc.scalar.activation(msg[:, :512], zA[:], Relu)
                    nc.vector.tensor_scalar_max(msg[:, 512:], zB[:], 0.0)
                    msgs.append(msg)
                    Sss.append(Ss)

                # dual-form aggregation: g2T[f,dst] += msg[e,f].T @ S[e,dst]
                # 4 passes of 2 feature-chunks; each chunk owns a PSUM bank.
                g2T = sg2T.tile([P, 8, P], bf16, tag="g2T")
                for p4 in range(4):
                    gA = gp.tile([P, 512], f32, tag="g")
                    gB = gp.tile([P, 512], f32, tag="g")
                    jA, jB = 2 * p4, 2 * p4 + 1
                    for t in range(nt):
                        nc.tensor.matmul(
                            gA[:, :P], msgs[t][:, jA * P : (jA + 1) * P],
                            Sss[t][:], start=(t == 0), stop=(t == nt - 1),
                        )
                        nc.tensor.matmul(
                            gB[:, :P], msgs[t][:, jB * P : (jB + 1) * P],
                            Sss[t][:], start=(t == 0), stop=(t == nt - 1),
                        )
                    nc.scalar.activation(g2T[:, jA], gA[:, :P], Copy)
                    nc.vector.tensor_copy(g2T[:, jB], gB[:, :P])

                # h2 = relu(g2 @ W2 + b2); then pg3 += T.T @ h2
                hps = []
                for half in range(2):
                    lo = half * 512
                    h2p = zp.tile([P, 512], f32, tag="z")
                    for j in range(8):
                        nc.tensor.matmul(
                            h2p[:], g2T[:, j], W2s[:, j, lo : lo + 512],
                            start=(j == 0), stop=False,
                        )
                    nc.tensor.matmul(
                        h2p[:], ones1[:1, :], b2s[:1, lo : lo + 512],
                        start=False, stop=True,
                    )
                    hps.append(h2p)
                h2b = sh2.tile([P, H], bf16, tag="h2b")
                nc.scalar.activation(h2b[:, :512], hps[0][:], Relu)
                nc.vector.tensor_scalar_max(h2b[:, 512:], hps[1][:], 0.0)
                for half in range(2):
                    lo = half * 512
                    cp = zp.tile([P, 512], f32, tag="z")
                    nc.tensor.matmul(
                        cp[:], Tt[:], h2b[:, lo : lo + 512], start=True, stop=True
                    )
                    nc.vector.tensor_add(
                        pg3s[:, lo : lo + 512], pg3s[:, lo : lo + 512], cp[:]
                    )

            nc.sync.dma_start(out_d[:], pg3s[:])

    nc.finalize()
    return nc


def kernel(x, W1, b1, W2, b2, W3, b3, Wlin, blin, edge_index, batch, num_graphs):
    import ml_dtypes
    from concourse.bass_utils import run_bass_kernel_spmd

    bf16 = ml_dtypes.bfloat16
    x = np.asarray(x, dtype=np.float32)
    W1 = np.asarray(W1, dtype=np.float32)
    b1 = np.asarray(b1, dtype=np.float32)
    W2 = np.asarray(W2, dtype=np.float32)
    b2 = np.asarray(b2, dtype=np.float32)
    W3 = np.asarray(W3, dtype=np.float32)
    b3 = np.asarray(b3, dtype=np.float32)
    Wlin = np.asarray(Wlin, dtype=np.float32)
    blin = np.asarray(blin, dtype=np.float32)

    agg1, aT, S, Tpad, cnt, T_w, TT, base_tile = _host_prep(x, edge_index, batch)

    nc = _build_device_program(TT, T_w, base_tile)

    W1b = np.zeros((4, H), dtype=np.float32)
    W1b[:2] = W1
    W1b[2] = b1
    W2r = np.ascontiguousarray(W2.reshape(8, P, H)).astype(bf16)
    b2r = b2.reshape(1, H).astype(bf16)
    W1b = W1b.astype(bf16)

    in_maps = [
        {
            "aT": np.ascontiguousarray(aT[c]),
            "S": np.ascontiguousarray(S[c].reshape(TT, P, P)),
            "T": np.ascontiguousarray(Tpad[c].reshape(NW, P, G)),
            "W1b": W1b,
            "W2": W2r,
            "b2": b2r,
        }
        for c in range(N_CORES)
    ]
    res = run_bass_kernel_spmd(nc, in_maps, core_ids=list(range(N_CORES)))
    global LAST_RESULTS
    LAST_RESULTS = res
    pg3 = np.zeros((G, H), dtype=np.float64)
    for r in res.results:
        pg3 += r["pg3"].astype(np.float64)
    pg3 = pg3.astype(np.float32)

    pooled = (pg3 @ W3 + cnt[:, None] * b3[None, :]) / np.maximum(cnt, 1.0)[:, None]
    out = pooled @ Wlin + blin[None, :]
    return out.astype(np.float32)


# revision 10
# speedup vs baseline: 5.4475x; 1.3725x over previous
"""GCN (3-layer, PyG-style) forward on 8 Trainium2 NeuronCores.

Math restructuring
------------------
reference:
  h1 = relu(Anorm @ x @ W1 + b1)          (Anorm includes self loops + sym norm)
  h2 = relu(Anorm @ h1 @ W2 + b2)
  h3 = Anorm @ h2 @ W3 + b3
  out = segment_mean(h3, batch) @ Wlin + blin

Because GCNConv aggregation and the weight matmul commute, and pooling is
linear, this is equivalent to:
  agg1 = Anorm @ x                        # [N,2]  (tiny -> host)
  msg_e = relu(norm_e * (agg1[src_e] @ W1 + b1))    # per-edge (norm>0 commutes
                                                    #  through relu)
  g2   = scatter-sum msg to dst           # exact one-hot matmul on device
  h2   = relu(g2 @ W2 + b2)               # dense matmul on device
  pg3[g] = sum_n T[n,g] * h2[n]           # T[n,g] = sum of norm over n's
                                          #  out-edges into graph g
  out  = ((pg3 @ W3 + cnt*b3)/max(cnt,1)) @ Wlin + blin   # [128,1024] -> host

Sharding: nodes are LPT bin-packed into 8 cores x 98 windows of 128 slots so
that each (core, window) bin holds ~638 incident edges (load-balanced).  Every
core runs the same program (SPMD) on its own edge arrays, padded to identical
tile counts.  Per-core output is a partial pg3 [128,1024]; the host sums them
(the "all-reduce").

Device-side structure:
 - all matmul operands bf16 (single-pass PE, FWL weight loads), PSUM fp32
 - L1 (K=4) matmuls are 4-way row-group packed via tile_position: aT and W1b
   are duplicated at SBUF base partitions {0,32,64,96} so the four matmuls of
   an edge-tile pair run concurrently in separate 32-row strips
 - aggregation runs in "dual form" (g2T[f,dst] += msg[e,f].T @ S[e,dst])
   chunk-wise, so no PE transposes are needed before the W2 matmul
 - L1 work for window w+1 is interleaved between the aggregation passes of
   window w, keeping every matmul's dependencies one window ahead (dense PE
   stream, HAM stays warm)
"""

import numpy as np

LAST_RESULTS = None  # set by kernel() for test harness introspection

N_NODES = 100000
N_EDGES = 400000
G = 128
FIN = 2
H = 1024
N_CORES = 8
P = 128
NW = 98                      # windows per core (98*128 = 12544 >= 12500 slots)
NBINS = N_CORES * NW


def _lpt_pack(wgt):
    """Assign each node to one of 784 (core,window) bins, balancing total
    edge weight per bin with a <=128 nodes/bin cap.  Returns (bin_of, slot_of).
    """
    import heapq

    n = len(wgt)
    order = np.argsort(-wgt, kind="stable")
    heap = [(0, 0, b) for b in range(NBINS)]
    heapq.heapify(heap)
    bin_of = np.empty(n, dtype=np.int64)
    slot_of = np.empty(n, dtype=np.int64)
    w_arr = wgt.tolist()
    for idx in order.tolist():
        while True:
            load, count, b = heapq.heappop(heap)
            if count < P:
                break
        bin_of[idx] = b
        slot_of[idx] = count
        heapq.heappush(heap, (load + w_arr[idx], count + 1, b))
    return bin_of, slot_of


def _host_prep(x, edge_index, batch):
    """All O(E) index work in numpy; returns per-core device arrays."""
    import ml_dtypes

    bf16 = ml_dtypes.bfloat16
    x = np.asarray(x, dtype=np.float32)
    ei = np.asarray(edge_index).astype(np.int64)
    batch = np.asarray(batch).astype(np.int64)
    n = N_NODES

    loops = np.arange(n, dtype=np.int64)
    row = np.concatenate([ei[0], loops])
    col = np.concatenate([ei[1], loops])

    deg = np.bincount(col, minlength=n).astype(np.float64)
    dis = np.where(deg > 0, 1.0 / np.sqrt(np.maximum(deg, 1.0)), 0.0)
    norm = dis[row] * dis[col]                     # fp64

    # layer-1 aggregation (FIN=2) on host
    agg1 = np.empty((n, FIN), dtype=np.float64)
    for f in range(FIN):
        agg1[:, f] = np.bincount(
            col, weights=norm * x[row, f].astype(np.float64), minlength=n
        )

    # ---- node -> (core, window, slot) via LPT packing on indegree+1 ----
    wgt = np.bincount(col, minlength=n)            # includes the self loop
    bin_raw, slot_of = _lpt_pack(wgt)
    # deal bins to (core, window) so similar loads share a window
    loads = np.zeros(NBINS, dtype=np.int64)
    np.add.at(loads, bin_raw, wgt)
    deal = np.argsort(-loads, kind="stable")       # deal[k] = raw bin id
    bin_rank = np.empty(NBINS, dtype=np.int64)
    bin_rank[deal] = np.arange(NBINS)
    rank = bin_rank[bin_raw]                       # 0..783, sorted by load
    node_w = rank // N_CORES                       # window 0..97
    node_c = rank % N_CORES                        # core 0..7

    # ---- edges ordered by (dst core, dst window) ----
    e_rank = rank[col]
    order = np.argsort(e_rank, kind="stable")
    row_s, col_s = row[order], col[order]
    norm_s = norm[order]
    rank_s = e_rank[order]
    c_s = rank_s % N_CORES
    w_s = rank_s // N_CORES

    cnts = np.bincount(e_rank, minlength=NBINS)    # indexed by rank = w*8 + c
    cw_load = cnts.reshape(NW, N_CORES).T          # [core, window]
    T_w = ((cw_load.max(axis=0) + P - 1) // P).astype(np.int64)   # per window
    base_tile = np.concatenate([[0], np.cumsum(T_w)])
    TT = int(base_tile[-1])
    if TT % 2:                                     # keep tile pairs aligned
        T_w[-1] += 1
        base_tile = np.concatenate([[0], np.cumsum(T_w)])
        TT = int(base_tile[-1])

    starts = np.concatenate([[0], np.cumsum(cnts)])
    idx_in_bin = np.arange(len(col_s)) - starts[rank_s]
    tile_g = base_tile[w_s] + idx_in_bin // P
    slot = tile_g * P + idx_in_bin % P

    # per-core device arrays (norm folded into aT; S is exact one-hot)
    aT = np.zeros((N_CORES, 4, TT * P), dtype=np.float32)
    S = np.zeros((N_CORES, TT * P, P), dtype=bf16)
    aT[c_s, 0, slot] = (agg1[row_s, 0] * norm_s).astype(np.float32)
    aT[c_s, 1, slot] = (agg1[row_s, 1] * norm_s).astype(np.float32)
    aT[c_s, 2, slot] = norm_s.astype(np.float32)
    S[c_s, slot, slot_of[col_s]] = bf16(1.0)

    # pair-interleaved + 2x duplicated aT layout for 4-way row-group packing:
    # rows 0-3 tile 2q / 4-7 tile 2q+1 / 8-11 dup of 0-3 / 12-15 dup of 4-7
    a3 = aT.reshape(N_CORES, 4, TT, P)
    ev = a3[:, :, 0::2, :].reshape(N_CORES, 4, -1)
    od = a3[:, :, 1::2, :].reshape(N_CORES, 4, -1)
    aT4 = np.concatenate([ev, od, ev, od], axis=1).astype(bf16)

    # ---- L3: T matrix rows permuted to node home slots ----
    gcol = batch[col]                              # graph of each edge's dst
    Tmat = np.bincount(
        row * G + gcol, weights=norm, minlength=n * G
    ).astype(np.float32).reshape(n, G)
    Tpad = np.zeros((N_CORES, NW * P, G), dtype=bf16)
    Tpad[node_c, node_w * P + slot_of] = Tmat.astype(bf16)

    cnt = np.bincount(batch, minlength=G).astype(np.float32)
    return aT4, S, Tpad, cnt, T_w, TT, base_tile


def _build_device_program(TT, T_w, base_tile, nw=NW):
    import concourse.mybir as mybir
    import concourse.tile as tile
    from concourse import bacc

    f32 = mybir.dt.float32
    bf16 = mybir.dt.bfloat16
    nc = bacc.Bacc(None, target_bir_lowering=False, debug=False)

    TQ = TT // 2                 # tile pairs
    aT_d = nc.dram_tensor("aT", [16, TQ * P], bf16, kind="ExternalInput")
    S_d = nc.dram_tensor("S", [TT, P, P], bf16, kind="ExternalInput")
    T_d = nc.dram_tensor("T", [NW, P, G], bf16, kind="ExternalInput")
    W1b_d = nc.dram_tensor("W1b", [4, H], bf16, kind="ExternalInput")
    W2_d = nc.dram_tensor("W2", [8, P, H], bf16, kind="ExternalInput")
    b2_d = nc.dram_tensor("b2", [1, H], bf16, kind="ExternalInput")
    out_d = nc.dram_tensor("pg3", [G, H], f32, kind="ExternalOutput")

    CHQ = 16                     # aT pairs per staged chunk
    n_chunks = (TQ + CHQ - 1) // CHQ

    with tile.TileContext(nc) as tc:
        with (
            tc.tile_pool(name="const", bufs=1) as cst,
            tc.tile_pool(name="sa", bufs=2) as sa,
            tc.tile_pool(name="sS", bufs=16) as sS,
            tc.tile_pool(name="smsg", bufs=16) as smsg,
            tc.tile_pool(name="sg2T", bufs=2) as sg2T,
            tc.tile_pool(name="sh2", bufs=2) as sh2,
            tc.tile_pool(name="sT", bufs=2) as sT,
            tc.tile_pool(name="zp", bufs=4, space="PSUM") as zp,
            tc.tile_pool(name="gp", bufs=4, space="PSUM") as gp,
        ):
            Relu = mybir.ActivationFunctionType.Relu
            Copy = mybir.ActivationFunctionType.Copy

            # W1b duplicated at base partitions 0/32/64/96
            W1bd = cst.tile([100, H], bf16, tag="W1bd")
            for g4 in range(4):
                nc.sync.dma_start(W1bd[g4 * 32 : g4 * 32 + 4, :], W1b_d[:])
            W2s = cst.tile([P, 8, H], bf16, tag="W2s")
            nc.sync.dma_start(W2s[:], W2_d[:].rearrange("c p f -> p c f"))
            b2s = cst.tile([1, H], bf16, tag="b2s")
            nc.sync.dma_start(b2s[:], b2_d[:])
            ones1 = cst.tile([1, P], bf16, tag="ones1")
            nc.vector.memset(ones1[:], 1.0)
            pg3s = cst.tile([G, H], f32, tag="pg3s")
            nc.vector.memset(pg3s[:], 0.0)

            chunks = {}          # chunk idx -> staged aT tile
            msg_of = {}          # global tile -> msg tile
            Ss_of = {}           # global tile -> one-hot S tile

            def stage_chunk(ci):
                if ci >= n_chunks or ci in chunks:
                    return
                t_ = sa.tile([100, CHQ * P], bf16, tag="aTc")
                lo = ci * CHQ * P
                hi = min((ci + 1) * CHQ * P, TQ * P)
                for g4 in range(4):
                    nc.sync.dma_start(
                        t_[g4 * 32 : g4 * 32 + 4, : hi - lo],
                        aT_d[g4 * 4 : g4 * 4 + 4, lo:hi],
                    )
                chunks[ci] = t_

            state = {"q": 0}

            def emit_pair():
                q = state["q"]
                if 2 * q >= TT:
                    return
                state["q"] = q + 1
                ci, off = q // CHQ, (q % CHQ) * P
                if q % CHQ == 0:
                    stage_chunk(ci + 1)
                aTc = chunks[ci]
                tg0, tg1 = 2 * q, 2 * q + 1
                for tg in (tg0, tg1):
                    Ss = sS.tile([P, P], bf16, tag="Ss")
                    nc.sync.dma_start(Ss[:], S_d[tg])
                    Ss_of[tg] = Ss
                zAe = zp.tile([P, 512], f32, tag="z")
                zAo = zp.tile([P, 512], f32, tag="z")
                zBe = zp.tile([P, 512], f32, tag="z")
                zBo = zp.tile([P, 512], f32, tag="z")
                sl = slice(off, off + P)
                nc.tensor.matmul(zAe[:], aTc[0:4, sl], W1bd[0:4, :512],
                                 start=True, stop=True, tile_position=(0, 0))
                nc.tensor.matmul(zAo[:], aTc[32:36, sl], W1bd[32:36, :512],
                                 start=True, stop=True, tile_position=(32, 0))
                nc.tensor.matmul(zBe[:], aTc[64:68, sl], W1bd[64:68, 512:],
                                 start=True, stop=True, tile_position=(64, 0))
                nc.tensor.matmul(zBo[:], aTc[96:100, sl], W1bd[96:100, 512:],
                                 start=True, stop=True, tile_position=(96, 0))
                m0 = smsg.tile([P, H], bf16, tag="msg")
                m1 = smsg.tile([P, H], bf16, tag="msg")
                nc.scalar.activation(m0[:, :512], zAe[:], Relu)
                nc.vector.tensor_scalar_max(m0[:, 512:], zBe[:], 0.0)
                nc.scalar.activation(m1[:, :512], zAo[:], Relu)
                nc.vector.tensor_scalar_max(m1[:, 512:], zBo[:], 0.0)
                msg_of[tg0], msg_of[tg1] = m0, m1

            def emit_pair_if(target):
                if 2 * state["q"] < min(target, TT):
                    emit_pair()

            # prologue: window 0's tiles
            stage_chunk(0)
            while 2 * state["q"] < int(base_tile[1]):
                emit_pair()

            for w in range(nw):
                Tt = sT.tile([P, G], bf16, tag="Tt")
                nc.sync.dma_start(Tt[:], T_d[w])
                nt = int(T_w[w])
                b0 = int(base_tile[w])
                target = int(base_tile[min(w + 2, nw)])

                # dual-form aggregation: g2T[f,dst] += msg[e,f].T @ S[e,dst]
                # 4 passes of 2 feature-chunks; each chunk owns a PSUM bank.
                g2T = sg2T.tile([P, 8, P], bf16, tag="g2T")
                for p4 in range(4):
                    gA = gp.tile([P, 512], f32, tag="g")
                    gB = gp.tile([P, 512], f32, tag="g")
                    jA, jB = 2 * p4, 2 * p4 + 1
                    for t in range(nt):
                        tg = b0 + t
                        nc.tensor.matmul(
                            gA[:, :P], msg_of[tg][:, jA * P : (jA + 1) * P],
                            Ss_of[tg][:], start=(t == 0), stop=(t == nt - 1),
                        )
                        nc.tensor.matmul(
                            gB[:, :P], msg_of[tg][:, jB * P : (jB + 1) * P],
                            Ss_of[tg][:], start=(t == 0), stop=(t == nt - 1),
                        )
                    nc.scalar.activation(g2T[:, jA], gA[:, :P], Copy)
                    nc.vector.tensor_copy(g2T[:, jB], gB[:, :P])
                    emit_pair_if(target)    # interleave next window's L1

                # h2 = relu(g2 @ W2 + b2); then pg3 += T.T @ h2
                hps = []
                for half in range(2):
                    lo = half * 512
                    h2p = zp.tile([P, 512], f32, tag="z")
                    for j in range(8):
                        nc.tensor.matmul(
                            h2p[:], g2T[:, j], W2s[:, j, lo : lo + 512],
                            start=(j == 0), stop=False,
                        )
                    nc.tensor.matmul(
                        h2p[:], ones1[:1, :], b2s[:1, lo : lo + 512],
                        start=False, stop=True,
                    )
                    hps.append(h2p)
                h2b = sh2.tile([P, H], bf16, tag="h2b")
                nc.scalar.activation(h2b[:, :512], hps[0][:], Relu)
                nc.vector.tensor_scalar_max(h2b[:, 512:], hps[1][:], 0.0)
                emit_pair_if(target)
                for half in range(2):
                    lo = half * 512
                    cp = zp.tile([P, 512], f32, tag="z")
                    nc.tensor.matmul(
                        cp[:], Tt[:], h2b[:, lo : lo + 512], start=True, stop=True
                    )
                    nc.vector.tensor_add(
                        pg3s[:, lo : lo + 512], pg3s[:, lo : lo + 512], cp[:]
                    )
                # drop consumed references so the dicts stay small
                for t in range(nt):
                    msg_of.pop(b0 + t, None)
                    Ss_of.pop(b0 + t, None)

            nc.sync.dma_start(out_d[:], pg3s[:])

    nc.finalize()
    return nc


def kernel(x, W1, b1, W2, b2, W3, b3, Wlin, blin, edge_index, batch, num_graphs):
    import ml_dtypes
    from concourse.bass_utils import run_bass_kernel_spmd

    bf16 = ml_dtypes.bfloat16
    x = np.asarray(x, dtype=np.float32)
    W1 = np.asarray(W1, dtype=np.float32)
    b1 = np.asarray(b1, dtype=np.float32)
    W2 = np.asarray(W2, dtype=np.float32)
    b2 = np.asarray(b2, dtype=np.float32)
    W3 = np.asarray(W3, dtype=np.float32)
    b3 = np.asarray(b3, dtype=np.float32)
    Wlin = np.asarray(Wlin, dtype=np.float32)
    blin = np.asarray(blin, dtype=np.float32)

    aT4, S, Tpad, cnt, T_w, TT, base_tile = _host_prep(x, edge_index, batch)

    nc = _build_device_program(TT, T_w, base_tile)

    W1b = np.zeros((4, H), dtype=np.float32)
    W1b[:2] = W1
    W1b[2] = b1
    W2r = np.ascontiguousarray(W2.reshape(8, P, H)).astype(bf16)
    b2r = b2.reshape(1, H).astype(bf16)
    W1b = W1b.astype(bf16)

    in_maps = [
        {
            "aT": np.ascontiguousarray(aT4[c]),
            "S": np.ascontiguousarray(S[c].reshape(TT, P, P)),
            "T": np.ascontiguousarray(Tpad[c].reshape(NW, P, G)),
            "W1b": W1b,
            "W2": W2r,
            "b2": b2r,
        }
        for c in range(N_CORES)
    ]
    res = run_bass_kernel_spmd(nc, in_maps, core_ids=list(range(N_CORES)))
    global LAST_RESULTS
    LAST_RESULTS = res
    pg3 = np.zeros((G, H), dtype=np.float64)
    for r in res.results:
        pg3 += r["pg3"].astype(np.float64)
    pg3 = pg3.astype(np.float32)

    pooled = (pg3 @ W3 + cnt[:, None] * b3[None, :]) / np.maximum(cnt, 1.0)[:, None]
    out = pooled @ Wlin + blin[None, :]
    return out.astype(np.float32)


# revision 13
# speedup vs baseline: 5.7617x; 1.0577x over previous
"""GCN (3-layer, PyG-style) forward on 8 Trainium2 NeuronCores.

Math restructuring
------------------
reference:
  h1 = relu(Anorm @ x @ W1 + b1)          (Anorm includes self loops + sym norm)
  h2 = relu(Anorm @ h1 @ W2 + b2)
  h3 = Anorm @ h2 @ W3 + b3
  out = segment_mean(h3, batch) @ Wlin + blin

Because GCNConv aggregation and the weight matmul commute, and pooling is
linear, this is equivalent to:
  agg1 = Anorm @ x                        # [N,2]  (tiny -> host)
  msg_e = relu(norm_e * (agg1[src_e] @ W1 + b1))    # per-edge (norm>0 commutes
                                                    #  through relu)
  g2   = scatter-sum msg to dst           # exact one-hot matmul on device
  h2   = relu(g2 @ W2 + b2)               # dense matmul on device
  pg3[g] = sum_n T[n,g] * h2[n]           # T[n,g] = sum of norm over n's
                                          #  out-edges into graph g
  out  = ((pg3 @ W3 + cnt*b3)/max(cnt,1)) @ Wlin + blin   # [128,1024] -> host

Sharding: nodes are LPT bin-packed into 8 cores x 98 windows of 128 slots so
that each (core, window) bin holds ~638 incident edges (load-balanced).  Every
core runs the same program (SPMD) on its own edge arrays, padded to identical
tile counts.  Per-core output is a partial pg3 [128,1024]; the host sums them
(the "all-reduce").

Device-side structure:
 - all matmul operands bf16 (single-pass PE, FWL weight loads), PSUM fp32
 - L1 (K=4) matmuls are 4-way row-group packed via tile_position: aT and W1b
   are duplicated at SBUF base partitions {0,32,64,96} so the four matmuls of
   an edge-tile pair run concurrently in separate 32-row strips
 - aggregation runs in "dual form" (g2T[f,dst] += msg[e,f].T @ S[e,dst])
   chunk-wise, so no PE transposes are needed before the W2 matmul
 - L1 work for window w+1 is interleaved between the aggregation passes of
   window w, keeping every matmul's dependencies one window ahead (dense PE
   stream, HAM stays warm)
"""

import numpy as np

LAST_RESULTS = None  # set by kernel() for test harness introspection

N_NODES = 100000
N_EDGES = 400000
G = 128
FIN = 2
H = 1024
N_CORES = 8
P = 128
NW = 98                      # windows per core (98*128 = 12544 >= 12500 slots)
NBINS = N_CORES * NW


def _lpt_pack(wgt):
    """Assign each node to one of 784 (core,window) bins, balancing total
    edge weight per bin with a <=128 nodes/bin cap.  Returns (bin_of, slot_of).
    """
    import heapq

    n = len(wgt)
    order = np.argsort(-wgt, kind="stable")
    heap = [(0, 0, b) for b in range(NBINS)]
    heapq.heapify(heap)
    bin_of = np.empty(n, dtype=np.int64)
    slot_of = np.empty(n, dtype=np.int64)
    w_arr = wgt.tolist()
    for idx in order.tolist():
        while True:
            load, count, b = heapq.heappop(heap)
            if count < P:
                break
        bin_of[idx] = b
        slot_of[idx] = count
        heapq.heappush(heap, (load + w_arr[idx], count + 1, b))
    return bin_of, slot_of


def _host_prep(x, edge_index, batch):
    """All O(E) index work in numpy; returns per-core device arrays."""
    import ml_dtypes

    bf16 = ml_dtypes.bfloat16
    x = np.asarray(x, dtype=np.float32)
    ei = np.asarray(edge_index).astype(np.int64)
    batch = np.asarray(batch).astype(np.int64)
    n = N_NODES

    loops = np.arange(n, dtype=np.int64)
    row = np.concatenate([ei[0], loops])
    col = np.concatenate([ei[1], loops])

    deg = np.bincount(col, minlength=n).astype(np.float64)
    dis = np.where(deg > 0, 1.0 / np.sqrt(np.maximum(deg, 1.0)), 0.0)
    norm = dis[row] * dis[col]                     # fp64

    # layer-1 aggregation (FIN=2) on host
    agg1 = np.empty((n, FIN), dtype=np.float64)
    for f in range(FIN):
        agg1[:, f] = np.bincount(
            col, weights=norm * x[row, f].astype(np.float64), minlength=n
        )

    # ---- node -> (core, window, slot) via LPT packing on indegree+1 ----
    wgt = np.bincount(col, minlength=n)            # includes the self loop
    bin_raw, slot_of = _lpt_pack(wgt)
    # deal bins to (core, window) so similar loads share a window
    loads = np.zeros(NBINS, dtype=np.int64)
    np.add.at(loads, bin_raw, wgt)
    deal = np.argsort(-loads, kind="stable")       # deal[k] = raw bin id
    bin_rank = np.empty(NBINS, dtype=np.int64)
    bin_rank[deal] = np.arange(NBINS)
    rank = bin_rank[bin_raw]                       # 0..783, sorted by load
    node_w = rank // N_CORES                       # window 0..97
    node_c = rank % N_CORES                        # core 0..7

    # ---- edges ordered by (dst core, dst window) ----
    e_rank = rank[col]
    order = np.argsort(e_rank, kind="stable")
    row_s, col_s = row[order], col[order]
    norm_s = norm[order]
    rank_s = e_rank[order]
    c_s = rank_s % N_CORES
    w_s = rank_s // N_CORES

    cnts = np.bincount(e_rank, minlength=NBINS)    # indexed by rank = w*8 + c
    cw_load = cnts.reshape(NW, N_CORES).T          # [core, window]
    T_w = ((cw_load.max(axis=0) + P - 1) // P).astype(np.int64)   # per window
    base_tile = np.concatenate([[0], np.cumsum(T_w)])
    TT = int(base_tile[-1])
    if TT % 2:                                     # keep tile pairs aligned
        T_w[-1] += 1
        base_tile = np.concatenate([[0], np.cumsum(T_w)])
        TT = int(base_tile[-1])

    starts = np.concatenate([[0], np.cumsum(cnts)])
    idx_in_bin = np.arange(len(col_s)) - starts[rank_s]
    tile_g = base_tile[w_s] + idx_in_bin // P
    slot = tile_g * P + idx_in_bin % P

    # per-core device arrays (norm folded into aT; S is exact one-hot)
    aT = np.zeros((N_CORES, 4, TT * P), dtype=np.float32)
    S = np.zeros((N_CORES, TT * P, P), dtype=bf16)
    aT[c_s, 0, slot] = (agg1[row_s, 0] * norm_s).astype(np.float32)
    aT[c_s, 1, slot] = (agg1[row_s, 1] * norm_s).astype(np.float32)
    aT[c_s, 2, slot] = norm_s.astype(np.float32)
    S[c_s, slot, slot_of[col_s]] = bf16(1.0)

    # pair-interleaved + 2x duplicated aT layout for 4-way row-group packing:
    # rows 0-3 tile 2q / 4-7 tile 2q+1 / 8-11 dup of 0-3 / 12-15 dup of 4-7
    a3 = aT.reshape(N_CORES, 4, TT, P)
    ev = a3[:, :, 0::2, :].reshape(N_CORES, 4, -1)
    od = a3[:, :, 1::2, :].reshape(N_CORES, 4, -1)
    aT4 = np.concatenate([ev, od, ev, od], axis=1).astype(bf16)

    # ---- L3: T matrix rows permuted to node home slots ----
    gcol = batch[col]                              # graph of each edge's dst
    Tmat = np.bincount(
        row * G + gcol, weights=norm, minlength=n * G
    ).astype(np.float32).reshape(n, G)
    Tpad = np.zeros((N_CORES, NW * P, G), dtype=bf16)
    Tpad[node_c, node_w * P + slot_of] = Tmat.astype(bf16)

    cnt = np.bincount(batch, minlength=G).astype(np.float32)
    return aT4, S, Tpad, cnt, T_w, TT, base_tile


def _build_device_program(TT, T_w, base_tile, nw=NW):
    import concourse.mybir as mybir
    import concourse.tile as tile
    from concourse import bacc

    f32 = mybir.dt.float32
    bf16 = mybir.dt.bfloat16
    nc = bacc.Bacc(None, target_bir_lowering=False, debug=False)

    TQ = TT // 2                 # tile pairs
    aT_d = nc.dram_tensor("aT", [16, TQ * P], bf16, kind="ExternalInput")
    S_d = nc.dram_tensor("S", [TT, P, P], bf16, kind="ExternalInput")
    T_d = nc.dram_tensor("T", [NW, P, G], bf16, kind="ExternalInput")
    W1b_d = nc.dram_tensor("W1b", [4, H], bf16, kind="ExternalInput")
    W2_d = nc.dram_tensor("W2", [8, P, H], bf16, kind="ExternalInput")
    b2_d = nc.dram_tensor("b2", [1, H], bf16, kind="ExternalInput")
    out_d = nc.dram_tensor("pg3", [G, H], f32, kind="ExternalOutput")

    CHQ = 16                     # aT pairs per staged chunk
    n_chunks = (TQ + CHQ - 1) // CHQ

    with tile.TileContext(nc) as tc:
        with (
            tc.tile_pool(name="const", bufs=1) as cst,
            tc.tile_pool(name="sa", bufs=2) as sa,
            tc.tile_pool(name="sS", bufs=16) as sS,
            tc.tile_pool(name="smsg", bufs=16) as smsg,
            tc.tile_pool(name="sg2T", bufs=2) as sg2T,
            tc.tile_pool(name="sh2", bufs=2) as sh2,
            tc.tile_pool(name="sT", bufs=2) as sT,
            tc.tile_pool(name="zp", bufs=4, space="PSUM") as zp,
            tc.tile_pool(name="gp", bufs=2, space="PSUM") as gp,
            tc.tile_pool(name="hp", bufs=2, space="PSUM") as hp,
        ):
            Relu = mybir.ActivationFunctionType.Relu
            Copy = mybir.ActivationFunctionType.Copy

            # W1b duplicated at base partitions 0/32/64/96
            W1bd = cst.tile([100, H], bf16, tag="W1bd")
            for g4 in range(4):
                nc.sync.dma_start(W1bd[g4 * 32 : g4 * 32 + 4, :], W1b_d[:])
            W2s = cst.tile([P, 8, H], bf16, tag="W2s")
            nc.sync.dma_start(W2s[:], W2_d[:].rearrange("c p f -> p c f"))
            b2s = cst.tile([1, H], bf16, tag="b2s")
            nc.sync.dma_start(b2s[:], b2_d[:])
            ones1 = cst.tile([1, P], bf16, tag="ones1")
            nc.vector.memset(ones1[:], 1.0)
            pg3s = cst.tile([G, H], f32, tag="pg3s")
            nc.vector.memset(pg3s[:], 0.0)

            chunks = {}          # chunk idx -> staged aT tile
            msg_of = {}          # global tile -> msg tile
            Ss_of = {}           # global tile -> one-hot S tile

            def stage_chunk(ci):
                if ci >= n_chunks or ci in chunks:
                    return
                t_ = sa.tile([100, CHQ * P], bf16, tag="aTc")
                lo = ci * CHQ * P
                hi = min((ci + 1) * CHQ * P, TQ * P)
                for g4 in range(4):
                    nc.sync.dma_start(
                        t_[g4 * 32 : g4 * 32 + 4, : hi - lo],
                        aT_d[g4 * 4 : g4 * 4 + 4, lo:hi],
                    )
                chunks[ci] = t_

            state = {"q": 0}

            def emit_pair():
                q = state["q"]
                if 2 * q >= TT:
                    return
                state["q"] = q + 1
                ci, off = q // CHQ, (q % CHQ) * P
                if q % CHQ == 0:
                    stage_chunk(ci + 1)
                aTc = chunks[ci]
                tg0, tg1 = 2 * q, 2 * q + 1
                for tg in (tg0, tg1):
                    Ss = sS.tile([P, P], bf16, tag="Ss")
                    nc.sync.dma_start(Ss[:], S_d[tg])
                    Ss_of[tg] = Ss
                zAe = zp.tile([P, 512], f32, tag="z")
                zAo = zp.tile([P, 512], f32, tag="z")
                zBe = zp.tile([P, 512], f32, tag="z")
                zBo = zp.tile([P, 512], f32, tag="z")
                sl = slice(off, off + P)
                nc.tensor.matmul(zAe[:], aTc[0:4, sl], W1bd[0:4, :512],
                                 start=True, stop=True, tile_position=(0, 0))
                nc.tensor.matmul(zAo[:], aTc[32:36, sl], W1bd[32:36, :512],
                                 start=True, stop=True, tile_position=(32, 0))
                nc.tensor.matmul(zBe[:], aTc[64:68, sl], W1bd[64:68, 512:],
                                 start=True, stop=True, tile_position=(64, 0))
                nc.tensor.matmul(zBo[:], aTc[96:100, sl], W1bd[96:100, 512:],
                                 start=True, stop=True, tile_position=(96, 0))
                m0 = smsg.tile([P, H], bf16, tag="msg")
                m1 = smsg.tile([P, H], bf16, tag="msg")
                nc.scalar.activation(m0[:, :512], zAe[:], Relu)
                nc.vector.tensor_scalar_max(m0[:, 512:], zBe[:], 0.0)
                nc.scalar.activation(m1[:, :512], zAo[:], Relu)
                nc.vector.tensor_scalar_max(m1[:, 512:], zBo[:], 0.0)
                msg_of[tg0], msg_of[tg1] = m0, m1

            def emit_pair_if(target):
                if 2 * state["q"] < min(target, TT):
                    emit_pair()

            # prologue: window 0's tiles
            stage_chunk(0)
            while 2 * state["q"] < int(base_tile[1]):
                emit_pair()

            for w in range(nw):
                Tt = sT.tile([P, G], bf16, tag="Tt")
                nc.sync.dma_start(Tt[:], T_d[w])
                nt = int(T_w[w])
                b0 = int(base_tile[w])
                target = int(base_tile[min(w + 2, nw)])

                # dual-form aggregation: g2T[f,dst] += msg[e,f].T @ S[e,dst]
                # 4 passes of 2 feature-chunks; each chunk owns a PSUM bank.
                g2T = sg2T.tile([P, 8, P], bf16, tag="g2T")
                for p4 in range(4):
                    gA = gp.tile([P, 512], f32, tag="g")
                    gB = gp.tile([P, 512], f32, tag="g")
                    jA, jB = 2 * p4, 2 * p4 + 1
                    for t in range(nt):
                        tg = b0 + t
                        nc.tensor.matmul(
                            gA[:, :P], msg_of[tg][:, jA * P : (jA + 1) * P],
                            Ss_of[tg][:], start=(t == 0), stop=(t == nt - 1),
                        )
                        nc.tensor.matmul(
                            gB[:, :P], msg_of[tg][:, jB * P : (jB + 1) * P],
                            Ss_of[tg][:], start=(t == 0), stop=(t == nt - 1),
                        )
                    nc.scalar.activation(g2T[:, jA], gA[:, :P], Copy)
                    nc.vector.tensor_copy(g2T[:, jB], gB[:, :P])
                    emit_pair_if(target)    # interleave next window's L1

                # h2 = relu(g2 @ W2 + b2); then pg3 += T.T @ h2
                hps = []
                for half in range(2):
                    lo = half * 512
                    h2p = hp.tile([P, 512], f32, tag="h")
                    for j in range(8):
                        nc.tensor.matmul(
                            h2p[:], g2T[:, j], W2s[:, j, lo : lo + 512],
                            start=(j == 0), stop=False,
                        )
                    nc.tensor.matmul(
                        h2p[:], ones1[:1, :], b2s[:1, lo : lo + 512],
                        start=False, stop=True,
                    )
                    hps.append(h2p)
                h2b = sh2.tile([P, H], bf16, tag="h2b")
                nc.scalar.activation(h2b[:, :512], hps[0][:], Relu)
                nc.vector.tensor_scalar_max(h2b[:, 512:], hps[1][:], 0.0)
                emit_pair_if(target)
                for half in range(2):
                    lo = half * 512
                    cp = hp.tile([P, 512], f32, tag="h")
                    nc.tensor.matmul(
                        cp[:], Tt[:], h2b[:, lo : lo + 512], start=True, stop=True
                    )
                    nc.vector.tensor_add(
                        pg3s[:, lo : lo + 512], pg3s[:, lo : lo + 512], cp[:]
                    )
                # drop consumed references so the dicts stay small
                for t in range(nt):
                    msg_of.pop(b0 + t, None)
                    Ss_of.pop(b0 + t, None)

            nc.sync.dma_start(out_d[:], pg3s[:])

    nc.finalize()
    return nc


def kernel(x, W1, b1, W2, b2, W3, b3, Wlin, blin, edge_index, batch, num_graphs):
    import ml_dtypes
    from concourse.bass_utils import run_bass_kernel_spmd

    bf16 = ml_dtypes.bfloat16
    x = np.asarray(x, dtype=np.float32)
    W1 = np.asarray(W1, dtype=np.float32)
    b1 = np.asarray(b1, dtype=np.float32)
    W2 = np.asarray(W2, dtype=np.float32)
    b2 = np.asarray(b2, dtype=np.float32)
    W3 = np.asarray(W3, dtype=np.float32)
    b3 = np.asarray(b3, dtype=np.float32)
    Wlin = np.asarray(Wlin, dtype=np.float32)
    blin = np.asarray(blin, dtype=np.float32)

    aT4, S, Tpad, cnt, T_w, TT, base_tile = _host_prep(x, edge_index, batch)

    nc = _build_device_program(TT, T_w, base_tile)

    W1b = np.zeros((4, H), dtype=np.float32)
    W1b[:2] = W1
    W1b[2] = b1
    W2r = np.ascontiguousarray(W2.reshape(8, P, H)).astype(bf16)
    b2r = b2.reshape(1, H).astype(bf16)
    W1b = W1b.astype(bf16)

    in_maps = [
        {
            "aT": np.ascontiguousarray(aT4[c]),
            "S": np.ascontiguousarray(S[c].reshape(TT, P, P)),
            "T": np.ascontiguousarray(Tpad[c].reshape(NW, P, G)),
            "W1b": W1b,
            "W2": W2r,
            "b2": b2r,
        }
        for c in range(N_CORES)
    ]
    res = run_bass_kernel_spmd(nc, in_maps, core_ids=list(range(N_CORES)))
    global LAST_RESULTS
    LAST_RESULTS = res
    pg3 = np.zeros((G, H), dtype=np.float64)
    for r in res.results:
        pg3 += r["pg3"].astype(np.float64)
    pg3 = pg3.astype(np.float32)

    pooled = (pg3 @ W3 + cnt[:, None] * b3[None, :]) / np.maximum(cnt, 1.0)[:, None]
    out = pooled @ Wlin + blin[None, :]
    return out.astype(np.float32)


# revision 20
# speedup vs baseline: 6.7089x; 1.1644x over previous
"""GCN (3-layer, PyG-style) forward on 8 Trainium2 NeuronCores.

Math restructuring
------------------
reference:
  h1 = relu(Anorm @ x @ W1 + b1)          (Anorm includes self loops + sym norm)
  h2 = relu(Anorm @ h1 @ W2 + b2)
  h3 = Anorm @ h2 @ W3 + b3
  out = segment_mean(h3, batch) @ Wlin + blin

Because GCNConv aggregation and the weight matmul commute, and pooling is
linear, this is equivalent to:
  agg1 = Anorm @ x                        # [N,2]  (tiny -> host)
  msg_e = relu(norm_e * (agg1[src_e] @ W1 + b1))    # per-edge (norm>0 commutes
                                                    #  through relu)
  g2   = scatter-sum msg to dst           # exact one-hot matmul on device
  h2   = relu(g2 @ W2 + b2)               # dense matmul on device
  pg3[g] = sum_n T[n,g] * h2[n]           # T[n,g] = sum of norm over n's
                                          #  out-edges into graph g
  out  = ((pg3 @ W3 + cnt*b3)/max(cnt,1)) @ Wlin + blin   # [128,1024] -> host

Sharding: nodes are LPT bin-packed into 8 cores x 98 windows of 128 slots so
that each (core, window) bin holds ~638 incident edges (load-balanced).  Every
core runs the same program (SPMD) on its own edge arrays, padded to identical
tile counts.  Per-core output is a partial pg3 [128,1024]; the host sums them
(the "all-reduce").

Device-side structure:
 - all matmul operands bf16 (single-pass PE, FWL weight loads), PSUM fp32
 - L1 (K=4) matmuls are 4-way row-group packed via tile_position: aT and W1b
   are duplicated at SBUF base partitions {0,32,64,96} so the four matmuls of
   an edge-tile pair run concurrently in separate 32-row strips
 - aggregation runs in "dual form" (g2T[f,dst] += msg[e,f].T @ S[e,dst])
   chunk-wise, so no PE transposes are needed before the W2 matmul
 - L1 work for window w+1 is interleaved between the aggregation passes of
   window w, keeping every matmul's dependencies one window ahead (dense PE
   stream, HAM stays warm)
"""

import numpy as np

LAST_RESULTS = None  # set by kernel() for test harness introspection

N_NODES = 100000
N_EDGES = 400000
G = 128
FIN = 2
H = 1024
N_CORES = 8
P = 128
NW = 98                      # windows per core (98*128 = 12544 >= 12500 slots)
NBINS = N_CORES * NW


def _lpt_pack(wgt):
    """Assign each node to one of 784 (core,window) bins, balancing total
    edge weight per bin with a <=128 nodes/bin cap.  Returns (bin_of, slot_of).
    """
    import heapq

    n = len(wgt)
    order = np.argsort(-wgt, kind="stable")
    heap = [(0, 0, b) for b in range(NBINS)]
    heapq.heapify(heap)
    bin_of = np.empty(n, dtype=np.int64)
    slot_of = np.empty(n, dtype=np.int64)
    w_arr = wgt.tolist()
    for idx in order.tolist():
        while True:
            load, count, b = heapq.heappop(heap)
            if count < P:
                break
        bin_of[idx] = b
        slot_of[idx] = count
        heapq.heappush(heap, (load + w_arr[idx], count + 1, b))
    return bin_of, slot_of


def _host_prep(x, edge_index, batch):
    """All O(E) index work in numpy; returns per-core device arrays."""
    import ml_dtypes

    bf16 = ml_dtypes.bfloat16
    x = np.asarray(x, dtype=np.float32)
    ei = np.asarray(edge_index).astype(np.int64)
    batch = np.asarray(batch).astype(np.int64)
    n = N_NODES

    loops = np.arange(n, dtype=np.int64)
    row = np.concatenate([ei[0], loops])
    col = np.concatenate([ei[1], loops])

    deg = np.bincount(col, minlength=n).astype(np.float64)
    dis = np.where(deg > 0, 1.0 / np.sqrt(np.maximum(deg, 1.0)), 0.0)
    norm = dis[row] * dis[col]                     # fp64

    # layer-1 aggregation (FIN=2) on host
    agg1 = np.empty((n, FIN), dtype=np.float64)
    for f in range(FIN):
        agg1[:, f] = np.bincount(
            col, weights=norm * x[row, f].astype(np.float64), minlength=n
        )

    # ---- node -> (core, window, slot) via LPT packing on indegree+1 ----
    wgt = np.bincount(col, minlength=n)            # includes the self loop
    bin_raw, slot_of = _lpt_pack(wgt)
    # deal bins to (core, window) so similar loads share a window
    loads = np.zeros(NBINS, dtype=np.int64)
    np.add.at(loads, bin_raw, wgt)
    deal = np.argsort(-loads, kind="stable")       # deal[k] = raw bin id
    bin_rank = np.empty(NBINS, dtype=np.int64)
    bin_rank[deal] = np.arange(NBINS)
    rank = bin_rank[bin_raw]                       # 0..783, sorted by load
    node_w = rank // N_CORES                       # window 0..97
    node_c = rank % N_CORES                        # core 0..7

    # ---- edges ordered by (dst core, dst window) ----
    e_rank = rank[col]
    order = np.argsort(e_rank, kind="stable")
    row_s, col_s = row[order], col[order]
    norm_s = norm[order]
    rank_s = e_rank[order]
    c_s = rank_s % N_CORES
    w_s = rank_s // N_CORES

    cnts = np.bincount(e_rank, minlength=NBINS)    # indexed by rank = w*8 + c
    cw_load = cnts.reshape(NW, N_CORES).T          # [core, window]
    T_w = ((cw_load.max(axis=0) + P - 1) // P).astype(np.int64)   # per window
    base_tile = np.concatenate([[0], np.cumsum(T_w)])
    TT = int(base_tile[-1])
    if TT % 2:                                     # keep tile pairs aligned
        T_w[-1] += 1
        base_tile = np.concatenate([[0], np.cumsum(T_w)])
        TT = int(base_tile[-1])

    starts = np.concatenate([[0], np.cumsum(cnts)])
    idx_in_bin = np.arange(len(col_s)) - starts[rank_s]
    tile_g = base_tile[w_s] + idx_in_bin // P
    slot = tile_g * P + idx_in_bin % P

    # per-core device arrays (norm folded into aT; S is exact one-hot)
    aT = np.zeros((N_CORES, 4, TT * P), dtype=np.float32)
    S = np.zeros((N_CORES, TT * P, P), dtype=bf16)
    aT[c_s, 0, slot] = (agg1[row_s, 0] * norm_s).astype(np.float32)
    aT[c_s, 1, slot] = (agg1[row_s, 1] * norm_s).astype(np.float32)
    aT[c_s, 2, slot] = norm_s.astype(np.float32)
    S[c_s, slot, slot_of[col_s]] = bf16(1.0)

    # pair-interleaved + 2x duplicated aT layout for 4-way row-group packing:
    # rows 0-3 tile 2q / 4-7 tile 2q+1 / 8-11 dup of 0-3 / 12-15 dup of 4-7
    a3 = aT.reshape(N_CORES, 4, TT, P)
    ev = a3[:, :, 0::2, :].reshape(N_CORES, 4, -1)
    od = a3[:, :, 1::2, :].reshape(N_CORES, 4, -1)
    aT4 = np.concatenate([ev, od, ev, od], axis=1).astype(bf16)

    # ---- L3: T matrix rows permuted to node home slots ----
    gcol = batch[col]                              # graph of each edge's dst
    Tmat = np.bincount(
        row * G + gcol, weights=norm, minlength=n * G
    ).astype(np.float32).reshape(n, G)
    Tpad = np.zeros((N_CORES, NW * P, G), dtype=bf16)
    Tpad[node_c, node_w * P + slot_of] = Tmat.astype(bf16)

    cnt = np.bincount(batch, minlength=G).astype(np.float32)
    return aT4, S, Tpad, cnt, T_w, TT, base_tile


def _build_device_program(TT, T_w, base_tile, nw=NW):
    import concourse.mybir as mybir
    import concourse.tile as tile
    from concourse import bacc

    f32 = mybir.dt.float32
    bf16 = mybir.dt.bfloat16
    fp8 = mybir.dt.float8e4
    nc = bacc.Bacc(None, target_bir_lowering=False, debug=False)

    TQ = TT // 2                 # tile pairs
    aT_d = nc.dram_tensor("aT", [16, TQ * P], bf16, kind="ExternalInput")
    S_d = nc.dram_tensor("S", [TT, P, P], bf16, kind="ExternalInput")
    T_d = nc.dram_tensor("T", [NW, P, G], bf16, kind="ExternalInput")
    W1b_d = nc.dram_tensor("W1b", [4, H], bf16, kind="ExternalInput")
    W2_d = nc.dram_tensor("W2", [8, P, H], fp8, kind="ExternalInput")
    b2_d = nc.dram_tensor("b2", [1, H], bf16, kind="ExternalInput")
    out_d = nc.dram_tensor("pg3", [G, H], f32, kind="ExternalOutput")

    CHQ = 16                     # aT pairs per staged chunk
    n_chunks = (TQ + CHQ - 1) // CHQ

    with tile.TileContext(nc) as tc:
        with (
            tc.tile_pool(name="const", bufs=1) as cst,
            tc.tile_pool(name="sa", bufs=2) as sa,
            tc.tile_pool(name="sS", bufs=16) as sS,
            tc.tile_pool(name="smsg", bufs=16) as smsg,
            tc.tile_pool(name="sg2T", bufs=2) as sg2T,
            tc.tile_pool(name="sh2", bufs=2) as sh2,
            tc.tile_pool(name="sT", bufs=2) as sT,
            tc.tile_pool(name="zp", bufs=4, space="PSUM") as zp,
            tc.tile_pool(name="gp", bufs=2, space="PSUM") as gp,
            tc.tile_pool(name="hp", bufs=2, space="PSUM") as hp,
        ):
            Relu = mybir.ActivationFunctionType.Relu
            Copy = mybir.ActivationFunctionType.Copy

            # W1b duplicated at base partitions 0/32/64/96
            W1bd = cst.tile([100, H], bf16, tag="W1bd")
            for g4 in range(4):
                nc.sync.dma_start(W1bd[g4 * 32 : g4 * 32 + 4, :], W1b_d[:])
            W2s = cst.tile([P, 8, H], fp8, tag="W2s")
            nc.sync.dma_start(W2s[:], W2_d[:].rearrange("c p f -> p c f"))
            b2s = cst.tile([1, H], bf16, tag="b2s")
            nc.sync.dma_start(b2s[:], b2_d[:])
            ones1 = cst.tile([1, P], bf16, tag="ones1")
            nc.vector.memset(ones1[:], 1.0)
            pg3s = cst.tile([G, H], f32, tag="pg3s")
            nc.vector.memset(pg3s[:], 0.0)

            chunks = {}          # chunk idx -> staged aT tile
            msg_of = {}          # global tile -> msg tile
            Ss_of = {}           # global tile -> one-hot S tile

            def stage_chunk(ci):
                if ci >= n_chunks or ci in chunks:
                    return
                t_ = sa.tile([100, CHQ * P], bf16, tag="aTc")
                lo = ci * CHQ * P
                hi = min((ci + 1) * CHQ * P, TQ * P)
                for g4 in range(4):
                    nc.sync.dma_start(
                        t_[g4 * 32 : g4 * 32 + 4, : hi - lo],
                        aT_d[g4 * 4 : g4 * 4 + 4, lo:hi],
                    )
                chunks[ci] = t_

            state = {"q": 0}

            def emit_pair():
                q = state["q"]
                if 2 * q >= TT:
                    return
                state["q"] = q + 1
                ci, off = q // CHQ, (q % CHQ) * P
                if q % CHQ == 0:
                    stage_chunk(ci + 1)
                aTc = chunks[ci]
                tg0, tg1 = 2 * q, 2 * q + 1
                for tg in (tg0, tg1):
                    Ss = sS.tile([P, P], bf16, tag="Ss")
                    nc.sync.dma_start(Ss[:], S_d[tg])
                    Ss_of[tg] = Ss
                zAe = zp.tile([P, 512], f32, tag="z")
                zAo = zp.tile([P, 512], f32, tag="z")
                zBe = zp.tile([P, 512], f32, tag="z")
                zBo = zp.tile([P, 512], f32, tag="z")
                sl = slice(off, off + P)
                nc.tensor.matmul(zAe[:], aTc[0:4, sl], W1bd[0:4, :512],
                                 start=True, stop=True, tile_position=(0, 0))
                nc.tensor.matmul(zAo[:], aTc[32:36, sl], W1bd[32:36, :512],
                                 start=True, stop=True, tile_position=(32, 0))
                nc.tensor.matmul(zBe[:], aTc[64:68, sl], W1bd[64:68, 512:],
                                 start=True, stop=True, tile_position=(64, 0))
                nc.tensor.matmul(zBo[:], aTc[96:100, sl], W1bd[96:100, 512:],
                                 start=True, stop=True, tile_position=(96, 0))
                m0 = smsg.tile([P, H], bf16, tag="msg")
                m1 = smsg.tile([P, H], bf16, tag="msg")
                nc.scalar.activation(m0[:, :512], zAe[:], Relu)
                nc.vector.tensor_scalar_max(m0[:, 512:], zBe[:], 0.0)
                nc.scalar.activation(m1[:, :512], zAo[:], Relu)
                nc.vector.tensor_scalar_max(m1[:, 512:], zBo[:], 0.0)
                msg_of[tg0], msg_of[tg1] = m0, m1

            def emit_pair_if(target):
                if 2 * state["q"] < min(target, TT):
                    emit_pair()

            # prologue: window 0's tiles
            stage_chunk(0)
            while 2 * state["q"] < int(base_tile[1]):
                emit_pair()

            for w in range(nw):
                Tt = sT.tile([P, G], bf16, tag="Tt")
                nc.sync.dma_start(Tt[:], T_d[w])
                nt = int(T_w[w])
                b0 = int(base_tile[w])
                target = int(base_tile[min(w + 2, nw)])

                # dual-form aggregation: g2T[f,dst] += msg[e,f].T @ S[e,dst]
                # 4 passes of 2 feature-chunks; each chunk owns a PSUM bank.
                g2T = sg2T.tile([P, 8, P], fp8, tag="g2T")
                for p4 in range(4):
                    gA = gp.tile([P, 512], f32, tag="g")
                    gB = gp.tile([P, 512], f32, tag="g")
                    jA, jB = 2 * p4, 2 * p4 + 1
                    for t in range(nt):
                        tg = b0 + t
                        nc.tensor.matmul(
                            gA[:, :P], msg_of[tg][:, jA * P : (jA + 1) * P],
                            Ss_of[tg][:], start=(t == 0), stop=(t == nt - 1),
                        )
                        nc.tensor.matmul(
                            gB[:, :P], msg_of[tg][:, jB * P : (jB + 1) * P],
                            Ss_of[tg][:], start=(t == 0), stop=(t == nt - 1),
                        )
                    nc.scalar.activation(g2T[:, jA], gA[:, :P], Copy, scale=8.0)
                    nc.vector.tensor_scalar_mul(g2T[:, jB], gB[:, :P], 8.0)
                    emit_pair_if(target)    # interleave next window's L1

                # h2 = relu(g2 @ W2 + b2); then pg3 += T.T @ h2
                hps = []
                for half in range(2):
                    lo = half * 512
                    h2p = hp.tile([P, 512], f32, tag="h")
                    for j2 in range(4):
                        nc.tensor.matmul(
                            h2p[:], g2T[:, 2 * j2 : 2 * j2 + 2, :],
                            W2s[:, 2 * j2 : 2 * j2 + 2, lo : lo + 512],
                            start=(j2 == 0), stop=False,
                            perf_mode=mybir.MatmulPerfMode.DoubleRow,
                        )
                    nc.tensor.matmul(
                        h2p[:], ones1[:1, :], b2s[:1, lo : lo + 512],
                        start=False, stop=True,
                    )
                    hps.append(h2p)
                h2b = sh2.tile([P, H], bf16, tag="h2b")
                nc.scalar.activation(h2b[:, :512], hps[0][:], Relu, scale=1.0 / 128)
                nc.vector.tensor_scalar(
                    h2b[:, 512:], hps[1][:], 1.0 / 128, 0.0,
                    op0=mybir.AluOpType.mult, op1=mybir.AluOpType.max,
                )
                emit_pair_if(target)
                for half in range(2):
                    lo = half * 512
                    cp = hp.tile([P, 512], f32, tag="h")
                    nc.tensor.matmul(
                        cp[:], Tt[:], h2b[:, lo : lo + 512], start=True, stop=True
                    )
                    nc.vector.tensor_add(
                        pg3s[:, lo : lo + 512], pg3s[:, lo : lo + 512], cp[:]
                    )
                # drop consumed references so the dicts stay small
                for t in range(nt):
                    msg_of.pop(b0 + t, None)
                    Ss_of.pop(b0 + t, None)

            nc.sync.dma_start(out_d[:], pg3s[:])

    nc.finalize()
    return nc


def kernel(x, W1, b1, W2, b2, W3, b3, Wlin, blin, edge_index, batch, num_graphs):
    import ml_dtypes
    from concourse.bass_utils import run_bass_kernel_spmd

    bf16 = ml_dtypes.bfloat16
    x = np.asarray(x, dtype=np.float32)
    W1 = np.asarray(W1, dtype=np.float32)
    b1 = np.asarray(b1, dtype=np.float32)
    W2 = np.asarray(W2, dtype=np.float32)
    b2 = np.asarray(b2, dtype=np.float32)
    W3 = np.asarray(W3, dtype=np.float32)
    b3 = np.asarray(b3, dtype=np.float32)
    Wlin = np.asarray(Wlin, dtype=np.float32)
    blin = np.asarray(blin, dtype=np.float32)

    aT4, S, Tpad, cnt, T_w, TT, base_tile = _host_prep(x, edge_index, batch)

    nc = _build_device_program(TT, T_w, base_tile)

    W1b = np.zeros((4, H), dtype=np.float32)
    W1b[:2] = W1
    W1b[2] = b1
    fp8 = ml_dtypes.float8_e4m3
    W2r = np.ascontiguousarray((W2 * 16.0).reshape(8, P, H)).astype(fp8)
    b2r = (b2 * 128.0).reshape(1, H).astype(bf16)
    W1b = W1b.astype(bf16)

    in_maps = [
        {
            "aT": np.ascontiguousarray(aT4[c]),
            "S": np.ascontiguousarray(S[c].reshape(TT, P, P)),
            "T": np.ascontiguousarray(Tpad[c].reshape(NW, P, G)),
            "W1b": W1b,
            "W2": W2r,
            "b2": b2r,
        }
        for c in range(N_CORES)
    ]
    res = run_bass_kernel_spmd(nc, in_maps, core_ids=list(range(N_CORES)))
    global LAST_RESULTS
    LAST_RESULTS = res
    pg3 = np.zeros((G, H), dtype=np.float64)
    for r in res.results:
        pg3 += r["pg3"].astype(np.float64)
    pg3 = pg3.astype(np.float32)

    pooled = (pg3 @ W3 + cnt[:, None] * b3[None, :]) / np.maximum(cnt, 1.0)[:, None]
    out = pooled @ Wlin + blin[None, :]
    return out.astype(np.float32)
